# revision 13
# baseline (speedup 1.0000x reference)
"""Trainium2 Bass kernel for CrossShotTransitionHamiltonian.

Math: H = H_idx (x) I_64 with H_idx the 16x16 cycle adjacency matrix, so
U_b = exp(-lam_b H) = M_b (x) I_64 with M_b = expm(-lam_b * H_idx) computed
exactly on the host from the (tiny) batch scalars lam_b.  Then

  rho_out[K,a,L,b] = sum_{k,l} M[K,k] M[L,l] rho[k,a,l,b]

i.e. viewing rho as a 16x16 grid of 64x64 latent blocks, the whole operator
is ONE dense contraction over the 256 (k,l) block-pair indices; the 4096
(a,b) latent positions ride along in the free dimension.  Per batch this is
a single [256x256] @ [256x4096] GEMM:

  out = T_b @ rho_pack,   T_b = kron(M_b, M_b) / trace_b  (256x256, symmetric)

with rho_pack[(k,l), (a,b)] = rho[k*64+a, l*64+b] packed on the host.
The trace normalization tr(U rho U) = tr(M^2 rho_blocks) is computed exactly
on the host from rho's block diagonals (0.5 MMAC) and folded into T_b.

rho_out is symmetric, so only the 136 upper-triangle block pairs (K<=L) are
computed and written back (128-row group + 8-row group); the host rebuilds
the lower triangle by transposing blocks.  Device work per batch: 32 bf16
matmuls (contraction 256 = 2x128 partitions) accumulated in fp32 PSUM, one
PSUM->SBUF evacuation with bf16 downcast (alternating scalar/vector), then
linear DMA out.  No transposes, no device trace pass.

Data-parallel over batch across 8 NeuronCores (4 batches/core), no
collectives.
"""

import numpy as np
import ml_dtypes

from concourse import bacc, mybir
from concourse import tile
from concourse.bass_utils import run_bass_kernel_spmd

NB = 4  # batch elements per core
NCORES = 8
D = 1024
F32 = mybir.dt.float32
BF16 = mybir.dt.float16
NPBF16 = np.float16

# upper-triangle block pairs, row-major: (0,0),(0,1),...,(0,15),(1,1),...
_PAIRS = [(K, L) for K in range(16) for L in range(K, 16)]  # 136
NPAIR = len(_PAIRS)
_COLIDX = np.array([K * 16 + L for (K, L) in _PAIRS])  # into 256


def _build_body(nc, tc, rho_d, t_d, out_d, nb=NB):
    from contextlib import ExitStack

    with ExitStack() as ctx:
        pool = ctx.enter_context(tc.tile_pool(name="work", bufs=1))
        pp = ctx.enter_context(tc.tile_pool(name="ps", bufs=1, space="PSUM"))

        # all batches' T matrices in one up-front DMA (host pre-transposed)
        tt = pool.tile([128, nb, 2, NPAIR], BF16, tag="tt", bufs=1, name="tt")
        nc.sync.dma_start(out=tt[:], in_=t_d)

        # every SBUF tile is unique (fits: 4*(2+1.06)MB + 0.28MB < 24MB), so
        # the only semaphore edges are true producer->consumer dependencies.
        # batch 0 arrives in 256KB quarters (compute starts ~2.5us earlier);
        # later batches in 1MB h-halves timed to land just before their use.
        rins = []
        for b in range(nb):
            rin = pool.tile([128, 2, 4096], BF16, tag=f"rin{b}", bufs=1,
                            name=f"rin{b}")
            if b == 0:
                for h in range(2):
                    for c in range(2):
                        for q in range(2):
                            sl = slice(2048 * h + 1024 * q,
                                       2048 * h + 1024 * (q + 1))
                            nc.sync.dma_start(
                                out=rin[:, c, sl], in_=rho_d[b, c, :, sl]
                            )
            else:
                for h in range(2):
                    sl = slice(2048 * h, 2048 * (h + 1))
                    nc.sync.dma_start(
                        out=rin[:, :, sl],
                        in_=rho_d[b, :, :, sl].rearrange("c p f -> p c f"),
                    )
            rins.append(rin)

        for b in range(nb):
            rin = rins[b]
            # g=0: upper pairs 0..127; g=1: upper pairs 128..135
            osb0 = pool.tile([128, 4096], BF16, tag=f"osb0_{b}", bufs=1,
                             name=f"osb0_{b}")
            osb1 = pool.tile([8, 4096], BF16, tag=f"osb1_{b}", bufs=1,
                             name=f"osb1_{b}")
            for g, (osb, np_) in enumerate(((osb0, 128), (osb1, 8))):
                for hh in range(2):
                    ps = pp.tile([128, 2048], F32, tag="pmm", bufs=2,
                                 name=f"ps{b}_{g}{hh}")
                    for c in range(2):
                        for q in range(4):
                            nc.tensor.matmul(
                                ps[:np_, 512 * q : 512 * (q + 1)],
                                lhsT=tt[:, b, c, 128 * g : 128 * g + np_],
                                rhs=rin[:, c, 2048 * hh + 512 * q :
                                        2048 * hh + 512 * (q + 1)],
                                start=(c == 0),
                                stop=(c == 1),
                            )
                    dst = osb[:, 2048 * hh : 2048 * (hh + 1)]
                    # big g0 evacs alternate scalar/vector; tiny g1 evacs
                    # both on scalar so each osb has one producer engine
                    if g == 0 and hh == 1:
                        nc.vector.tensor_copy(dst, ps[:np_])
                    else:
                        nc.scalar.copy(out=dst, in_=ps[:np_])
                    # stream each evacuated g0 half out right away so only
                    # the tiny g1 write remains after the last evacuation
                    if g == 0:
                        nc.gpsimd.dma_start(
                            out=out_d[b, 0:128, 2048 * hh : 2048 * (hh + 1)],
                            in_=dst,
                        )
            nc.gpsimd.dma_start(out=out_d[b, 128:NPAIR], in_=osb1[:])


def build_nc(nb=NB):
    nc = bacc.Bacc(
        "TRN2",
        target_bir_lowering=False,
        debug=False,
        enable_asserts=False,
        num_devices=NCORES,
    )
    rho_d = nc.dram_tensor("rho_p", (nb, 2, 128, 4096), BF16,
                           kind="ExternalInput").ap()
    t_d = nc.dram_tensor("tmat", (128, nb, 2, NPAIR), BF16,
                         kind="ExternalInput").ap()
    out_d = nc.dram_tensor("out", (nb, NPAIR, 4096), BF16,
                           kind="ExternalOutput").ap()

    with tile.TileContext(nc) as tc:
        _build_body(nc, tc, rho_d, t_d, out_d, nb=nb)
    nc.compile()
    return nc


# ---------------- host-side parameter prep ----------------

def _host_mats(rho, t, w1, b1, w2, b2):
    """lam -> M=expm(-lam*Hidx); trace tr(M^2 rho_blocks); T=kron(M,M)/tr."""
    x = t.astype(np.float64)[:, None]
    h = x @ w1.astype(np.float64).T + b1.astype(np.float64)
    h = h / (1.0 + np.exp(-h))  # silu
    lam = 0.1 * np.tanh(h @ w2.astype(np.float64).T + b2.astype(np.float64))[:, 0]

    k = np.arange(16)
    S = np.zeros((16, 16))
    S[(k + 1) % 16, k] = 1.0
    w_eig, V = np.linalg.eigh(S + S.T)
    E = np.exp(-lam[:, None] * w_eig[None, :])  # (B,16)
    M = np.einsum("ik,bk,jk->bij", V, E, V)  # (B,16,16)
    M2 = np.einsum("bij,bjk->bik", M, M)

    # tr(U rho U) = sum_{k,l} M2[k,l] * sum_a rho[k*64+a, l*64+a]
    B = M.shape[0]
    rr = rho.reshape(B, 16, 64, 16, 64)
    c = np.einsum("bkala->bkl", rr, optimize=True)
    tr = np.einsum("bkl,bkl->b", c.astype(np.float64), M2)
    tr = np.maximum(tr, 1e-8)

    # T[(k,l),(K,L)] = M[k,K]*M[l,L] / tr; keep only upper-pair columns
    T = np.einsum("bkK,blL->bklKL", M, M).reshape(B, 256, 256)
    T = T[:, :, _COLIDX] / tr[:, None, None]  # (B, 256, 136)
    # device layout: tmat[p, b, c, m] = T[b, c*128+p, m] (per-core slice)
    Tn = T.reshape(B, 2, 128, NPAIR).transpose(2, 0, 1, 3)
    return np.ascontiguousarray(Tn.astype(NPBF16))


_CACHE = {}


def _prep_in_maps(rho, t, w1, b1, w2, b2):
    rho = np.asarray(rho, dtype=np.float32)
    B = rho.shape[0]
    tmat = _host_mats(rho, np.asarray(t), np.asarray(w1), np.asarray(b1),
                      np.asarray(w2), np.asarray(b2))
    # rho_pack[b, (k,l), (a,b)] = rho[b, k*64+a, l*64+b]
    rp = rho.reshape(B, 16, 64, 16, 64).transpose(0, 1, 3, 2, 4)
    rp = np.ascontiguousarray(rp.reshape(B, 2, 128, 4096).astype(NPBF16))

    in_maps = []
    for c in range(NCORES):
        sl = slice(NB * c, NB * (c + 1))
        in_maps.append({
            "rho_p": rp[sl],
            "tmat": np.ascontiguousarray(tmat[:, sl]),
        })
    return in_maps


def _unpack_out(res):
    outs = [res.results[c]["out"] for c in range(NCORES)]
    outU = np.concatenate(outs, axis=0).astype(np.float32)  # (B,136,4096)
    B = outU.shape[0]
    full = np.empty((B, 16, 16, 64, 64), np.float32)
    iu = (np.array([p[0] for p in _PAIRS]), np.array([p[1] for p in _PAIRS]))
    full[:, iu[0], iu[1]] = outU.reshape(B, NPAIR, 64, 64)
    strict = iu[0] != iu[1]
    full[:, iu[1][strict], iu[0][strict]] = (
        full[:, iu[0][strict], iu[1][strict]].transpose(0, 1, 3, 2)
    )
    out = full.transpose(0, 1, 3, 2, 4).reshape(B, D, D)
    return np.ascontiguousarray(out)


def kernel(rho, t, w1, b1, w2, b2, H):
    in_maps = _prep_in_maps(rho, t, w1, b1, w2, b2)
    if "nc" not in _CACHE:
        _CACHE["nc"] = build_nc()
    nc = _CACHE["nc"]

    last_err = None
    for attempt in range(3):
        try:
            res = run_bass_kernel_spmd(nc, in_maps, core_ids=list(range(NCORES)))
            break
        except Exception as e:  # transient device-unrecoverable faults heal on retry
            last_err = e
            import time as _time

            _time.sleep(5.0)
    else:
        raise last_err
    return _unpack_out(res)
